# revision 14
# baseline (speedup 1.0000x reference)
"""Trainium2 Bass kernel for nn_Cache_28071906246843 (retrieval_knn).

reference semantics:
    q = h_t[cache_words]                         # [C, D] gather
    dist = sqrt(sum((cache_h - q)**2, -1))       # [C]
    vals = exp(dist / 32.0)                      # [C]
    cache_p = segment_sum(vals, cache_words, V)  # [V]
    out = log_softmax(cache_p[None, :])          # [1, V]

v7 design (all-pairs fp8 matmul + on-device one-hot selection):
    dist^2_i = ||ch_i||^2 + ||w_{r(i)}||^2 - 2 ch_i . w_{r(i)}
Both norms are host-precomputed; the device computes ONLY the selected
-2*ch.w dot per element.  Cache elements are sorted by word id and split
into 8 shards of 16384; per supertile of 256 sorted elements the <=128
distinct h_t rows (scaled by -2, cast to fp8e4m3 on host along with ch,
both pre-transposed to contraction-major blocks) meet in 16 fp8 matmuls
producing the all-pairs [256 elem, nd word] dot in PSUM.  Selection of
each element's own word runs fully on-device: a persistent fp32 iota row
plus a tiny per-element rel-index tile feed one fused DVE
scalar_tensor_tensor per (supertile, half): mask = (iota == rel),
product = mask * psum, accum_out = the selected dot — no one-hot mask
ever touches HBM (v5 spent ~1.8 MB/core on it).  Word blocks are packed
per-supertile to the max distinct count across cores (SPMD shares one
program), supertiles go in batches of 4-8 with small edge batches for
fast pipeline fill/drain, and dsel streams out in chunks.  The kernel is
HBM-bandwidth-bound: ~23 MB/core at the ~358 GB/s per-core cap.
sqrt/exp/segment-sum/log_softmax run on the host (tiny O(C)+O(V) work).
The v4 dedup-gather kernel is kept as a fallback in case a supertile
exceeds 128 distinct words.
"""

import sys

import numpy as np

if "/opt/trn_rl_repo" not in sys.path:
    sys.path.insert(0, "/opt/trn_rl_repo")

import ml_dtypes

import concourse.bass as bass
import concourse.tile as tile
from concourse import bacc, mybir
from concourse.bass_utils import run_bass_kernel_spmd

V, D, C = 50257, 1024, 131072
NCORES = 8
CSH = C // NCORES  # 16384 elements per core
P = 128            # SBUF partitions
NT = CSH // P      # 128 element-tiles per core
SMOOTH = 32.0

SUP = 2            # element-tiles per supertile
NSUP = NT // SUP   # 64 supertiles per core
SUPW = SUP * P     # 256 elements per supertile
NCH = D // P       # 8 contraction chunks

FP8 = ml_dtypes.float8_e4m3


def build_nc_v7(plan, ndmax, woff_s, wtot, nsup: int = NSUP) -> bass.Bass:
    """All-pairs dot kernel; one loop iteration per batch of supertiles.
    plan[i] = (s0, B): batch i covers supertiles [s0, s0+B).
      chb:     [128, nsup*8*256] fp8, chb[p][(s,c,u)] = ch[s*256+u, c*128+p]
               (partition-outermost: each batch is one fully-contiguous
               per-partition column slice -> optimal DMA descriptors)
      wob:     [128, wtot] fp8; supertile s owns cols [woff_s[s], woff_s[s+1])
               holding [c][j < ndmax[s]] of -2*ht rows
      relt:    [128, 2*nsup] fp32; relt[m][2s+g] = word slot of elem (s,g,m)
    PE:  psum[m, (h,g), j] = -2 ch.w all-pairs (elements stationary, fp8)
    DVE: 3 bulk ops per batch: oh = (iota == rel) via stride-0 broadcast
         APs (bf16 in, fp8 mask out), tmp = oh * psum (fp16 out), dsel
         cols = reduce_X(tmp) -- 16-bit streams run DVE at 2x.  PSUM
         slots are zeroed once at start so masked 0 * stale stays finite.
    DMAs ride BOTH HWDGE rings with ~equal bytes per batch, each ring in
    consumption order (sync: ch half 1 + words half 1; scalar: ch half 2
    + words half 2) -- a single ring serializes descriptor generation and
    completion receipts and tops out well below the fabric rate.
    """
    nc = bacc.Bacc(
        "TRN2", target_bir_lowering=False, debug=False, num_devices=NCORES
    )
    CHF = NCH * SUPW
    chb = nc.dram_tensor(
        "chb", [P, nsup * CHF], mybir.dt.float8e4, kind="ExternalInput"
    )
    wob = nc.dram_tensor("wob", [P, wtot], mybir.dt.float8e4, kind="ExternalInput")
    relt = nc.dram_tensor(
        "relt", [P, SUP * nsup], mybir.dt.bfloat16, kind="ExternalInput"
    )
    dsel = nc.dram_tensor(
        "dsel", [P, SUP * nsup], mybir.dt.float32, kind="ExternalOutput"
    )

    PSB = 2  # psum pool depth (a steady-state B=8 tile = 4 banks)
    chb_ap = chb.ap()  # [128, nsup*CHF]
    wob_ap = wob.ap()  # [128, wtot]

    with tile.TileContext(nc) as tc:
        with (
            tc.tile_pool(name="io", bufs=6) as io,
            tc.tile_pool(name="tmpp", bufs=2) as tmpp,
            tc.tile_pool(name="psum", bufs=PSB, space="PSUM") as psum,
            tc.tile_pool(name="persist", bufs=1) as persist,
        ):
            dsel_sb = persist.tile([P, SUP * nsup], mybir.dt.float32)
            rel_sb = persist.tile([P, SUP * nsup], mybir.dt.bfloat16)
            iota_sb = persist.tile([P, P], mybir.dt.bfloat16)
            nc.gpsimd.iota(
                iota_sb[:, :],
                pattern=[[1, P]],
                base=0,
                channel_multiplier=0,
                allow_small_or_imprecise_dtypes=True,
            )
            nc.scalar.dma_start(out=rel_sb[:], in_=relt.ap())
            # zero both PSUM slots once: afterwards every stale byte a
            # masked read may touch is 0.0 or an old finite fp8 dot, so
            # 0 * stale can never produce NaN
            for _ in range(PSB):
                pt_init = psum.tile([P, 16, P], mybir.dt.float32, tag="pt")
                nc.vector.memset(pt_init[:], 0.0)
            out_done = 0

            for i, (s0, B) in enumerate(plan):
                off0 = int(woff_s[s0])
                wcols = int(woff_s[s0 + B] - woff_s[s0])
                nb = int(max(ndmax[s0 : s0 + B]))
                ch_sb = io.tile([P, B, NCH, SUPW], mybir.dt.float8e4, tag="ch")
                w_sb = io.tile([P, wcols], mybir.dt.float8e4, tag="w")
                B0 = max(1, B // 2)
                if B > B0:
                    wmid = int(woff_s[s0 + B0])  # split words by supertile
                else:
                    wmid = off0 + (NCH // 2) * int(ndmax[s0])  # by chunk
                nc.sync.dma_start(
                    out=ch_sb[:, 0:B0, :, :],
                    in_=chb_ap[:, s0 * CHF : (s0 + B0) * CHF],
                )
                nc.sync.dma_start(
                    out=w_sb[:, 0 : wmid - off0],
                    in_=wob_ap[:, off0:wmid],
                )
                if B > B0:
                    nc.scalar.dma_start(
                        out=ch_sb[:, B0:B, :, :],
                        in_=chb_ap[:, (s0 + B0) * CHF : (s0 + B) * CHF],
                    )
                nc.scalar.dma_start(
                    out=w_sb[:, wmid - off0 : wcols],
                    in_=wob_ap[:, wmid : off0 + wcols],
                )

                pt = psum.tile([P, B * SUP, P], mybir.dt.float32, tag="pt")
                for h in range(B):
                    n = int(ndmax[s0 + h])
                    off = int(woff_s[s0 + h] - woff_s[s0])
                    for g in range(SUP):
                        for c in range(NCH):
                            nc.tensor.matmul(
                                out=pt[:, h * SUP + g, 0:n],
                                lhsT=ch_sb[:, h, c, g * P : (g + 1) * P],
                                rhs=w_sb[:, off + c * n : off + (c + 1) * n],
                                start=(c == 0),
                                stop=(c == NCH - 1),
                            )

                # oh[m, (h,g), w] = (iota[w] == rel[m, 2*(s0+h)+g])
                G = B * SUP
                oh = tmpp.tile([P, G, P], mybir.dt.float8e4, tag="oh")
                iota_sl = iota_sb[:, 0:nb]
                iota_b = bass.AP(
                    tensor=iota_sl.tensor,
                    offset=iota_sl.offset,
                    ap=[iota_sl.ap[0], [0, G], iota_sl.ap[1]],
                )
                rel_sl = rel_sb[:, SUP * s0 : SUP * s0 + G]
                rel_b = bass.AP(
                    tensor=rel_sl.tensor,
                    offset=rel_sl.offset,
                    ap=[rel_sl.ap[0], rel_sl.ap[1], [0, nb]],
                )
                nc.vector.tensor_tensor(
                    out=oh[:, :, 0:nb],
                    in0=iota_b,
                    in1=rel_b,
                    op=mybir.AluOpType.is_equal,
                )
                tmp = tmpp.tile([P, G, P], mybir.dt.float16, tag="tmp")
                nc.vector.tensor_tensor(
                    out=tmp[:, :, 0:nb],
                    in0=pt[:, :, 0:nb],
                    in1=oh[:, :, 0:nb],
                    op=mybir.AluOpType.mult,
                )
                nc.vector.tensor_reduce(
                    out=dsel_sb[:, SUP * s0 : SUP * (s0 + B)],
                    in_=tmp[:, :, 0:nb],
                    axis=mybir.AxisListType.X,
                    op=mybir.AluOpType.add,
                )

                # stream finished dsel columns out early (on the lighter
                # scalar ring, so they don't queue behind ch transfers) and
                # keep the final post-reduce DMA tiny
                done = SUP * (s0 + B)
                if done - out_done >= 48 or i >= len(plan) - 3:
                    nc.scalar.dma_start(
                        out=dsel.ap()[:, out_done:done],
                        in_=dsel_sb[:, out_done:done],
                    )
                    out_done = done
    nc.compile()
    return nc


def make_plan():
    batches = [1, 1, 2, 4] + [8] * 6 + [4, 2, 1, 1]
    plan, s0 = [], 0
    for b in batches:
        plan.append((s0, b))
        s0 += b
    assert s0 == NSUP
    return plan


def prep_v7(h_t, ch_sorted, cw_sorted):
    """Host-side block building for v7.  Returns None if any supertile has
    more than 128 distinct words (fall back to v4 then)."""
    S = NCORES * NSUP  # 512 supertiles total
    seg = cw_sorted.reshape(S, SUPW)
    widx = np.empty((S, P), np.int64)
    rel = np.empty((S, SUPW), np.int64)
    nd = np.empty(S, np.int64)
    for s in range(S):
        uw, r = np.unique(seg[s], return_inverse=True)
        if len(uw) > P:
            return None
        nd[s] = len(uw)
        widx[s, : len(uw)] = uw
        widx[s, len(uw):] = uw[-1]
        rel[s] = r
    # SPMD: all cores share one program, so pad each supertile's word count
    # to the max across cores
    ndmax = nd.reshape(NCORES, NSUP).max(axis=0)  # [NSUP]

    plan = make_plan()
    woff_s = np.zeros(NSUP + 1, np.int64)
    woff_s[1:] = np.cumsum(NCH * ndmax)
    wtot = int(woff_s[-1])

    ht8 = (-2.0 * h_t).astype(FP8)
    ch8 = ch_sorted.astype(FP8)

    # chb[core, p, (s, c, u)] = ch8[core*CSH + s*256+u, c*128+p]
    # (partition-outermost so each batch is one contiguous column slice)
    chb = np.ascontiguousarray(
        ch8.reshape(NCORES, NSUP, SUPW, NCH, P).transpose(0, 4, 1, 3, 2)
    ).reshape(NCORES, P, NSUP * NCH * SUPW)
    # wb[s, p, c, j] = ht8[widx[s, j], c*128+p]
    wb = ht8[widx].reshape(S, P, NCH, P).transpose(0, 3, 2, 1)  # [S, p, c, j]
    wb4 = wb.reshape(NCORES, NSUP, P, NCH, P)
    wob = np.empty((NCORES, P, wtot), FP8)
    for s in range(NSUP):
        n = int(ndmax[s])
        wob[:, :, int(woff_s[s]) : int(woff_s[s + 1])] = (
            wb4[:, s, :, :, :n].reshape(NCORES, P, NCH * n)
        )
    # relt[core, m, 2s+g] = rel index of element (s, g, m)
    relf = np.ascontiguousarray(
        rel.reshape(NCORES, NSUP, SUP, P).transpose(0, 3, 1, 2)
    ).reshape(NCORES, P, NSUP * SUP).astype(ml_dtypes.bfloat16)

    hn2 = np.einsum("ij,ij->i", h_t, h_t, dtype=np.float64)
    cn2 = np.einsum("ij,ij->i", ch_sorted, ch_sorted, dtype=np.float64)
    b = cn2 + hn2[cw_sorted]  # [C] norm part of dist^2, in sorted order
    return chb, wob, relf, b, plan, ndmax, woff_s, wtot


def make_in_maps_v7(chb, wob, relf):
    return [
        {"chb": chb[c], "wob": wob[c], "relt": relf[c]} for c in range(NCORES)
    ]


# ---------------------------------------------------------------------------
# v4 fallback (dedup gather + TensorE expand/subtract + ScalarE square)
# ---------------------------------------------------------------------------


def build_nc_v4(nt: int = NT, v: int = V, d: int = D) -> bass.Bass:
    nsup = nt // SUP
    nc = bacc.Bacc(
        "TRN2", target_bir_lowering=False, debug=False, num_devices=NCORES
    )
    ht = nc.dram_tensor("ht", [v, d], mybir.dt.float32, kind="ExternalInput")
    ch = nc.dram_tensor("ch", [nt * P, d], mybir.dt.float16, kind="ExternalInput")
    widx = nc.dram_tensor("widx", [P, nsup], mybir.dt.int32, kind="ExternalInput")
    nsel = nc.dram_tensor("nsel", [nt, P, P], mybir.dt.float16, kind="ExternalInput")
    ident = nc.dram_tensor("ident", [P, P], mybir.dt.float16, kind="ExternalInput")
    vals = nc.dram_tensor("vals", [P, nt], mybir.dt.float32, kind="ExternalOutput")

    ch_ap = ch.ap()
    nsel_ap = nsel.ap()

    with tile.TileContext(nc) as tc:
        with (
            tc.tile_pool(name="io", bufs=4) as io,
            tc.tile_pool(name="wpool", bufs=3) as wpool,
            tc.tile_pool(name="spool", bufs=4) as spool,
            tc.tile_pool(name="psum", bufs=4, space="PSUM") as psum,
            tc.tile_pool(name="scratch", bufs=2) as scratch,
            tc.tile_pool(name="persist", bufs=1) as persist,
        ):
            widx_sb = persist.tile([P, nsup], mybir.dt.int32)
            nc.sync.dma_start(out=widx_sb[:], in_=widx.ap())
            ident_sb = persist.tile([P, P], mybir.dt.float16)
            nc.sync.dma_start(out=ident_sb[:], in_=ident.ap())
            vals_sb = persist.tile([P, nt], mybir.dt.float32)
            d2_all = persist.tile([P, nt], mybir.dt.float32)

            for s in range(nsup):
                w_fp = wpool.tile([P, d], mybir.dt.float16, tag="wfp")
                nc.gpsimd.indirect_dma_start(
                    out=w_fp[:],
                    out_offset=None,
                    in_=ht.ap(),
                    in_offset=bass.IndirectOffsetOnAxis(
                        ap=widx_sb[:, s : s + 1], axis=0
                    ),
                )
                ch_sb = io.tile([P, SUP, d], mybir.dt.float16, tag="ch")
                ch_src = bass.AP(
                    tensor=ch_ap.tensor,
                    offset=s * SUPW * d,
                    ap=[[d, P], [P * d, SUP], [1, d]],
                )
                nc.sync.dma_start(out=ch_sb[:], in_=ch_src)
                ns_sb = spool.tile([P, SUP, P], mybir.dt.float16, tag="nsel")
                ns_src = bass.AP(
                    tensor=nsel_ap.tensor,
                    offset=s * SUP * P * P,
                    ap=[[P, P], [P * P, SUP], [1, P]],
                )
                nc.sync.dma_start(out=ns_sb[:], in_=ns_src)

                q_psums = []
                for k in range(SUP):
                    q_psum = psum.tile([P, d], mybir.dt.float32, tag="q")
                    q_psums.append(q_psum)
                    for h in range(0, d, 512):
                        nc.tensor.matmul(
                            out=q_psum[:, h : h + 512],
                            lhsT=ns_sb[:, k, :],
                            rhs=w_fp[:, h : h + 512],
                            start=True,
                            stop=(k == 1),
                        )
                for h in range(0, d, 512):
                    nc.tensor.matmul(
                        out=q_psums[0][:, h : h + 512],
                        lhsT=ident_sb[:],
                        rhs=ch_sb[:, 0, h : h + 512],
                        start=False,
                        stop=True,
                    )
                d_sb = io.tile([P, d], mybir.dt.float32, tag="dsb")
                nc.vector.tensor_tensor(
                    out=d_sb[:],
                    in0=ch_sb[:, 1, :],
                    in1=q_psums[1][:],
                    op=mybir.AluOpType.add,
                )
                t0 = SUP * s
                sq_tile = scratch.tile([P, d], mybir.dt.float32, tag="sq")
                nc.scalar.activation(
                    out=sq_tile[:],
                    in_=q_psums[0][:],
                    func=mybir.ActivationFunctionType.Square,
                    accum_out=d2_all[:, t0 : t0 + 1],
                )
                sq_tile2 = scratch.tile([P, d], mybir.dt.float32, tag="sq2")
                nc.scalar.activation(
                    out=sq_tile2[:],
                    in_=d_sb[:],
                    func=mybir.ActivationFunctionType.Square,
                    accum_out=d2_all[:, t0 + 1 : t0 + 2],
                )

            dist_all = persist.tile([P, nt], mybir.dt.float32)
            nc.scalar.activation(
                out=dist_all[:],
                in_=d2_all[:],
                func=mybir.ActivationFunctionType.Sqrt,
            )
            nc.scalar.activation(
                out=vals_sb[:],
                in_=dist_all[:],
                func=mybir.ActivationFunctionType.Exp,
                scale=1.0 / SMOOTH,
            )
            nc.sync.dma_start(out=vals.ap(), in_=vals_sb[:])
    nc.compile()
    return nc


def prep_v4(cw_sorted):
    widx_all, nsel_all = [], []
    neye = -np.eye(P, dtype=np.float16)
    for c in range(NCORES):
        shard = cw_sorted[c * CSH : (c + 1) * CSH]
        widx = np.empty((NSUP, P), np.int32)
        nsel = np.empty((NT, P, P), np.float16)
        for s in range(NSUP):
            seg = shard[s * SUPW : (s + 1) * SUPW]
            uw = np.unique(seg)
            if len(uw) > P:
                return None
            widx[s, : len(uw)] = uw
            widx[s, len(uw) :] = uw[-1]
            rel = np.searchsorted(uw, seg).reshape(SUP, P)
            for k in range(SUP):
                nsel[SUP * s + k] = neye[:, rel[k]]
        widx_all.append(np.ascontiguousarray(widx.T))
        nsel_all.append(nsel)
    return widx_all, nsel_all


def make_in_maps_v4(h_t, ch_sorted, widx_all, nsel_all):
    ident = np.eye(P, dtype=np.float16)
    in_maps = []
    for c in range(NCORES):
        sl = slice(c * CSH, (c + 1) * CSH)
        in_maps.append(
            {
                "ht": h_t,
                "ch": ch_sorted[sl].astype(np.float16),
                "widx": widx_all[c],
                "nsel": nsel_all[c],
                "ident": ident,
            }
        )
    return in_maps


def finish_on_host(vals_sorted, cw_sorted):
    """segment-sum + log_softmax (tiny O(C)+O(V) work)."""
    p = np.bincount(cw_sorted, weights=vals_sorted.astype(np.float64), minlength=V)
    m = p.max()
    lse = m + np.log(np.exp(p - m).sum())
    return (p - lse).astype(np.float32)[None, :]


def _prep(h_t, cache_h, cache_words):
    h_t = np.ascontiguousarray(np.asarray(h_t), dtype=np.float32)
    cache_h = np.ascontiguousarray(np.asarray(cache_h), dtype=np.float32)
    cw = np.asarray(cache_words).astype(np.int32)
    order = np.argsort(cw, kind="stable")
    return h_t, cache_h[order], cw[order]


def run_device(h_t, ch_sorted, cw_sorted, force_v1=False, verbose=False):
    """Compile + run the SPMD program; returns per-element vals (sorted order)."""
    import time as _time

    _t0 = _time.time()
    v7 = prep_v7(h_t, ch_sorted, cw_sorted)
    if v7 is not None:
        chb, wob, relf, b, plan, ndmax, woff_s, wtot = v7
        nc = build_nc_v7(plan, ndmax, woff_s, wtot)
        in_maps = make_in_maps_v7(chb, wob, relf)
        if verbose:
            mb = (chb.nbytes / NCORES + wob[0].nbytes + relf[0].nbytes) / 1e6
            print(f"[run_device] build+prep(v7): {_time.time() - _t0:.1f}s "
                  f"({mb:.1f} MB/core, wtot={wtot})")
        _t1 = _time.time()
        res = run_bass_kernel_spmd(nc, in_maps, core_ids=list(range(NCORES)))
        if verbose:
            print(f"[run_device] compile+exec: {_time.time() - _t1:.1f}s")
        # dsel[p, t] = selected -2*dot for element t*128+p (per core)
        dsel = np.concatenate(
            [r["dsel"].T.reshape(-1) for r in res.results]
        ).astype(np.float64)
        d2 = np.maximum(b + dsel, 0.0)
        return np.exp(np.sqrt(d2) / SMOOTH)

    v4 = prep_v4(cw_sorted)
    assert v4 is not None, "both v7 and v4 prep failed"
    nc = build_nc_v4()
    in_maps = make_in_maps_v4(h_t, ch_sorted, *v4)
    if verbose:
        print(f"[run_device] build+prep(v4): {_time.time() - _t0:.1f}s")
    _t1 = _time.time()
    res = run_bass_kernel_spmd(nc, in_maps, core_ids=list(range(NCORES)))
    if verbose:
        print(f"[run_device] compile+exec: {_time.time() - _t1:.1f}s")
    return np.concatenate([r["vals"].T.reshape(-1) for r in res.results])


def kernel(h_t, cache_h, cache_words):
    h_t, ch_sorted, cw_sorted = _prep(h_t, cache_h, cache_words)
    vals_sorted = run_device(h_t, ch_sorted, cw_sorted)
    return finish_on_host(vals_sorted, cw_sorted)


# revision 16
# speedup vs baseline: 1.0537x; 1.0537x over previous
"""Trainium2 Bass kernel for nn_Cache_28071906246843 (retrieval_knn).

reference semantics:
    q = h_t[cache_words]                         # [C, D] gather
    dist = sqrt(sum((cache_h - q)**2, -1))       # [C]
    vals = exp(dist / 32.0)                      # [C]
    cache_p = segment_sum(vals, cache_words, V)  # [V]
    out = log_softmax(cache_p[None, :])          # [1, V]

v7 design (all-pairs fp8 matmul + on-device one-hot selection):
    dist^2_i = ||ch_i||^2 + ||w_{r(i)}||^2 - 2 ch_i . w_{r(i)}
Both norms are host-precomputed; the device computes ONLY the selected
-2*ch.w dot per element.  Cache elements are sorted by word id and split
into 8 shards of 16384; per supertile of 256 sorted elements the <=128
distinct h_t rows (scaled by -2, cast to fp8e4m3 on host along with ch,
both pre-transposed to contraction-major blocks) meet in 16 fp8 matmuls
producing the all-pairs [256 elem, nd word] dot in PSUM.  Selection of
each element's own word runs fully on-device: a persistent fp32 iota row
plus a tiny per-element rel-index tile feed one fused DVE
scalar_tensor_tensor per (supertile, half): mask = (iota == rel),
product = mask * psum, accum_out = the selected dot — no one-hot mask
ever touches HBM (v5 spent ~1.8 MB/core on it).  Word blocks are packed
per-supertile to the max distinct count across cores (SPMD shares one
program), supertiles go in batches of 4-8 with small edge batches for
fast pipeline fill/drain, and dsel streams out in chunks.  The kernel is
HBM-bandwidth-bound: ~23 MB/core at the ~358 GB/s per-core cap.
sqrt/exp/segment-sum/log_softmax run on the host (tiny O(C)+O(V) work).
The v4 dedup-gather kernel is kept as a fallback in case a supertile
exceeds 128 distinct words.
"""

import sys

import numpy as np

if "/opt/trn_rl_repo" not in sys.path:
    sys.path.insert(0, "/opt/trn_rl_repo")

import ml_dtypes

import concourse.bass as bass
import concourse.tile as tile
from concourse import bacc, mybir
from concourse.bass_utils import run_bass_kernel_spmd

V, D, C = 50257, 1024, 131072
NCORES = 8
CSH = C // NCORES  # 16384 elements per core
P = 128            # SBUF partitions
NT = CSH // P      # 128 element-tiles per core
SMOOTH = 32.0

SUP = 2            # element-tiles per supertile
NSUP = NT // SUP   # 64 supertiles per core
SUPW = SUP * P     # 256 elements per supertile
NCH = D // P       # 8 contraction chunks

FP8 = ml_dtypes.float8_e4m3


def build_nc_v7(plan, ndmax, woff_s, wtot, nsup: int = NSUP) -> bass.Bass:
    """All-pairs dot kernel; one loop iteration per batch of supertiles.
    plan[i] = (s0, B): batch i covers supertiles [s0, s0+B).
      chb:     [128, nsup*8*256] fp8, chb[p][(s,c,u)] = ch[s*256+u, c*128+p]
               (partition-outermost: each batch is one fully-contiguous
               per-partition column slice -> optimal DMA descriptors)
      wob:     [128, wtot] fp8; supertile s owns cols [woff_s[s], woff_s[s+1])
               holding [c][j < ndmax[s]] of -2*ht rows
      relt:    [128, 2*nsup] fp32; relt[m][2s+g] = word slot of elem (s,g,m)
    PE:  psum[m, (h,g), j] = -2 ch.w all-pairs (elements stationary, fp8)
    DVE: 3 bulk ops per batch: oh = (iota == rel) via stride-0 broadcast
         APs (bf16 in, fp8 mask out), tmp = oh * psum (fp16 out), dsel
         cols = reduce_X(tmp) -- 16-bit streams run DVE at 2x.  PSUM
         slots are zeroed once at start so masked 0 * stale stays finite.
    DMAs ride BOTH HWDGE rings with ~equal bytes per batch, each ring in
    consumption order (sync: ch half 1 + words half 1; scalar: ch half 2
    + words half 2) -- a single ring serializes descriptor generation and
    completion receipts and tops out well below the fabric rate.
    """
    nc = bacc.Bacc(
        "TRN2", target_bir_lowering=False, debug=False, num_devices=NCORES
    )
    CHF = NCH * SUPW
    chb = nc.dram_tensor(
        "chb", [P, nsup * CHF], mybir.dt.float8e4, kind="ExternalInput"
    )
    wob = nc.dram_tensor("wob", [P, wtot], mybir.dt.float8e4, kind="ExternalInput")
    relt = nc.dram_tensor(
        "relt", [P, SUP * nsup], mybir.dt.bfloat16, kind="ExternalInput"
    )
    dsel = nc.dram_tensor(
        "dsel", [P, SUP * nsup], mybir.dt.float32, kind="ExternalOutput"
    )

    PSB = 2  # psum pool depth (a steady-state B=8 tile = 4 banks)
    chb_ap = chb.ap()  # [128, nsup*CHF]
    wob_ap = wob.ap()  # [128, wtot]

    with tile.TileContext(nc) as tc:
        with (
            tc.tile_pool(name="io", bufs=6) as io,
            tc.tile_pool(name="tmpp", bufs=2) as tmpp,
            tc.tile_pool(name="psum", bufs=PSB, space="PSUM") as psum,
            tc.tile_pool(name="persist", bufs=1) as persist,
        ):
            dsel_sb = persist.tile([P, SUP * nsup], mybir.dt.float32)
            rel_sb = persist.tile([P, SUP * nsup], mybir.dt.bfloat16)
            iota_sb = persist.tile([P, P], mybir.dt.bfloat16)
            nc.gpsimd.iota(
                iota_sb[:, :],
                pattern=[[1, P]],
                base=0,
                channel_multiplier=0,
                allow_small_or_imprecise_dtypes=True,
            )
            # rel rides the gpsimd SWDGE queue: off both HWDGE rings, so
            # its ~2us completion receipt never delays the ramp batches
            nc.gpsimd.dma_start(out=rel_sb[:], in_=relt.ap())
            # zero both PSUM slots once: afterwards every stale byte a
            # masked read may touch is 0.0 or an old finite fp8 dot, so
            # 0 * stale can never produce NaN
            for _ in range(PSB):
                pt_init = psum.tile([P, 16, P], mybir.dt.float32, tag="pt")
                nc.vector.memset(pt_init[:], 0.0)
            out_done = 0

            for i, (s0, B) in enumerate(plan):
                off0 = int(woff_s[s0])
                wcols = int(woff_s[s0 + B] - woff_s[s0])
                nb = int(max(ndmax[s0 : s0 + B]))
                ch_sb = io.tile([P, B, NCH, SUPW], mybir.dt.float8e4, tag="ch")
                w_sb = io.tile([P, wcols], mybir.dt.float8e4, tag="w")
                B0 = max(1, B // 2)
                if B > B0:
                    wmid = int(woff_s[s0 + B0])  # split words by supertile
                else:
                    wmid = off0 + (NCH // 2) * int(ndmax[s0])  # by chunk
                nc.sync.dma_start(
                    out=ch_sb[:, 0:B0, :, :],
                    in_=chb_ap[:, s0 * CHF : (s0 + B0) * CHF],
                )
                nc.sync.dma_start(
                    out=w_sb[:, 0 : wmid - off0],
                    in_=wob_ap[:, off0:wmid],
                )
                if B > B0:
                    nc.scalar.dma_start(
                        out=ch_sb[:, B0:B, :, :],
                        in_=chb_ap[:, (s0 + B0) * CHF : (s0 + B) * CHF],
                    )
                nc.scalar.dma_start(
                    out=w_sb[:, wmid - off0 : wcols],
                    in_=wob_ap[:, wmid : off0 + wcols],
                )

                pt = psum.tile([P, B * SUP, P], mybir.dt.float32, tag="pt")
                for h in range(B):
                    n = int(ndmax[s0 + h])
                    off = int(woff_s[s0 + h] - woff_s[s0])
                    for g in range(SUP):
                        for c in range(NCH):
                            nc.tensor.matmul(
                                out=pt[:, h * SUP + g, 0:n],
                                lhsT=ch_sb[:, h, c, g * P : (g + 1) * P],
                                rhs=w_sb[:, off + c * n : off + (c + 1) * n],
                                start=(c == 0),
                                stop=(c == NCH - 1),
                            )

                # oh[m, (h,g), w] = (iota[w] == rel[m, 2*(s0+h)+g])
                G = B * SUP
                oh = tmpp.tile([P, G, P], mybir.dt.float8e4, tag="oh")
                iota_sl = iota_sb[:, 0:nb]
                iota_b = bass.AP(
                    tensor=iota_sl.tensor,
                    offset=iota_sl.offset,
                    ap=[iota_sl.ap[0], [0, G], iota_sl.ap[1]],
                )
                rel_sl = rel_sb[:, SUP * s0 : SUP * s0 + G]
                rel_b = bass.AP(
                    tensor=rel_sl.tensor,
                    offset=rel_sl.offset,
                    ap=[rel_sl.ap[0], rel_sl.ap[1], [0, nb]],
                )
                nc.vector.tensor_tensor(
                    out=oh[:, :, 0:nb],
                    in0=iota_b,
                    in1=rel_b,
                    op=mybir.AluOpType.is_equal,
                )
                tmp = tmpp.tile([P, G, P], mybir.dt.float16, tag="tmp")
                nc.vector.tensor_tensor(
                    out=tmp[:, :, 0:nb],
                    in0=pt[:, :, 0:nb],
                    in1=oh[:, :, 0:nb],
                    op=mybir.AluOpType.mult,
                )
                nc.vector.tensor_reduce(
                    out=dsel_sb[:, SUP * s0 : SUP * (s0 + B)],
                    in_=tmp[:, :, 0:nb],
                    axis=mybir.AxisListType.X,
                    op=mybir.AluOpType.add,
                )

                # stream finished dsel columns out early (on the lighter
                # scalar ring, so they don't queue behind ch transfers) and
                # keep the final post-reduce DMA tiny
                done = SUP * (s0 + B)
                if done - out_done >= 48 or i >= len(plan) - 3:
                    nc.scalar.dma_start(
                        out=dsel.ap()[:, out_done:done],
                        in_=dsel_sb[:, out_done:done],
                    )
                    out_done = done
    nc.compile()
    return nc


def make_plan():
    # Few, large batches: every batch boundary costs ~2-3us of serialized
    # DMA-completion latency when the pipeline can't run ahead (ramp and
    # taper), so tiny fill/drain batches lose more to latency than they
    # save in transfer granularity.  Modest taper (4,2,2) keeps the
    # post-last-byte PE drain short without B=1 latency stalls.
    batches = [4, 4] + [8] * 6 + [4, 2, 2]
    plan, s0 = [], 0
    for b in batches:
        plan.append((s0, b))
        s0 += b
    assert s0 == NSUP
    return plan


def prep_v7(h_t, ch_sorted, cw_sorted):
    """Host-side block building for v7.  Returns None if any supertile has
    more than 128 distinct words (fall back to v4 then)."""
    S = NCORES * NSUP  # 512 supertiles total
    seg = cw_sorted.reshape(S, SUPW)
    widx = np.empty((S, P), np.int64)
    rel = np.empty((S, SUPW), np.int64)
    nd = np.empty(S, np.int64)
    for s in range(S):
        uw, r = np.unique(seg[s], return_inverse=True)
        if len(uw) > P:
            return None
        nd[s] = len(uw)
        widx[s, : len(uw)] = uw
        widx[s, len(uw):] = uw[-1]
        rel[s] = r
    # SPMD: all cores share one program, so pad each supertile's word count
    # to the max across cores
    ndmax = nd.reshape(NCORES, NSUP).max(axis=0)  # [NSUP]

    plan = make_plan()
    woff_s = np.zeros(NSUP + 1, np.int64)
    woff_s[1:] = np.cumsum(NCH * ndmax)
    wtot = int(woff_s[-1])

    ht8 = (-2.0 * h_t).astype(FP8)
    ch8 = ch_sorted.astype(FP8)

    # chb[core, p, (s, c, u)] = ch8[core*CSH + s*256+u, c*128+p]
    # (partition-outermost so each batch is one contiguous column slice)
    chb = np.ascontiguousarray(
        ch8.reshape(NCORES, NSUP, SUPW, NCH, P).transpose(0, 4, 1, 3, 2)
    ).reshape(NCORES, P, NSUP * NCH * SUPW)
    # wb[s, p, c, j] = ht8[widx[s, j], c*128+p]
    wb = ht8[widx].reshape(S, P, NCH, P).transpose(0, 3, 2, 1)  # [S, p, c, j]
    wb4 = wb.reshape(NCORES, NSUP, P, NCH, P)
    wob = np.empty((NCORES, P, wtot), FP8)
    for s in range(NSUP):
        n = int(ndmax[s])
        wob[:, :, int(woff_s[s]) : int(woff_s[s + 1])] = (
            wb4[:, s, :, :, :n].reshape(NCORES, P, NCH * n)
        )
    # relt[core, m, 2s+g] = rel index of element (s, g, m)
    relf = np.ascontiguousarray(
        rel.reshape(NCORES, NSUP, SUP, P).transpose(0, 3, 1, 2)
    ).reshape(NCORES, P, NSUP * SUP).astype(ml_dtypes.bfloat16)

    hn2 = np.einsum("ij,ij->i", h_t, h_t, dtype=np.float64)
    cn2 = np.einsum("ij,ij->i", ch_sorted, ch_sorted, dtype=np.float64)
    b = cn2 + hn2[cw_sorted]  # [C] norm part of dist^2, in sorted order
    return chb, wob, relf, b, plan, ndmax, woff_s, wtot


def make_in_maps_v7(chb, wob, relf):
    return [
        {"chb": chb[c], "wob": wob[c], "relt": relf[c]} for c in range(NCORES)
    ]


# ---------------------------------------------------------------------------
# v4 fallback (dedup gather + TensorE expand/subtract + ScalarE square)
# ---------------------------------------------------------------------------


def build_nc_v4(nt: int = NT, v: int = V, d: int = D) -> bass.Bass:
    nsup = nt // SUP
    nc = bacc.Bacc(
        "TRN2", target_bir_lowering=False, debug=False, num_devices=NCORES
    )
    ht = nc.dram_tensor("ht", [v, d], mybir.dt.float32, kind="ExternalInput")
    ch = nc.dram_tensor("ch", [nt * P, d], mybir.dt.float16, kind="ExternalInput")
    widx = nc.dram_tensor("widx", [P, nsup], mybir.dt.int32, kind="ExternalInput")
    nsel = nc.dram_tensor("nsel", [nt, P, P], mybir.dt.float16, kind="ExternalInput")
    ident = nc.dram_tensor("ident", [P, P], mybir.dt.float16, kind="ExternalInput")
    vals = nc.dram_tensor("vals", [P, nt], mybir.dt.float32, kind="ExternalOutput")

    ch_ap = ch.ap()
    nsel_ap = nsel.ap()

    with tile.TileContext(nc) as tc:
        with (
            tc.tile_pool(name="io", bufs=4) as io,
            tc.tile_pool(name="wpool", bufs=3) as wpool,
            tc.tile_pool(name="spool", bufs=4) as spool,
            tc.tile_pool(name="psum", bufs=4, space="PSUM") as psum,
            tc.tile_pool(name="scratch", bufs=2) as scratch,
            tc.tile_pool(name="persist", bufs=1) as persist,
        ):
            widx_sb = persist.tile([P, nsup], mybir.dt.int32)
            nc.sync.dma_start(out=widx_sb[:], in_=widx.ap())
            ident_sb = persist.tile([P, P], mybir.dt.float16)
            nc.sync.dma_start(out=ident_sb[:], in_=ident.ap())
            vals_sb = persist.tile([P, nt], mybir.dt.float32)
            d2_all = persist.tile([P, nt], mybir.dt.float32)

            for s in range(nsup):
                w_fp = wpool.tile([P, d], mybir.dt.float16, tag="wfp")
                nc.gpsimd.indirect_dma_start(
                    out=w_fp[:],
                    out_offset=None,
                    in_=ht.ap(),
                    in_offset=bass.IndirectOffsetOnAxis(
                        ap=widx_sb[:, s : s + 1], axis=0
                    ),
                )
                ch_sb = io.tile([P, SUP, d], mybir.dt.float16, tag="ch")
                ch_src = bass.AP(
                    tensor=ch_ap.tensor,
                    offset=s * SUPW * d,
                    ap=[[d, P], [P * d, SUP], [1, d]],
                )
                nc.sync.dma_start(out=ch_sb[:], in_=ch_src)
                ns_sb = spool.tile([P, SUP, P], mybir.dt.float16, tag="nsel")
                ns_src = bass.AP(
                    tensor=nsel_ap.tensor,
                    offset=s * SUP * P * P,
                    ap=[[P, P], [P * P, SUP], [1, P]],
                )
                nc.sync.dma_start(out=ns_sb[:], in_=ns_src)

                q_psums = []
                for k in range(SUP):
                    q_psum = psum.tile([P, d], mybir.dt.float32, tag="q")
                    q_psums.append(q_psum)
                    for h in range(0, d, 512):
                        nc.tensor.matmul(
                            out=q_psum[:, h : h + 512],
                            lhsT=ns_sb[:, k, :],
                            rhs=w_fp[:, h : h + 512],
                            start=True,
                            stop=(k == 1),
                        )
                for h in range(0, d, 512):
                    nc.tensor.matmul(
                        out=q_psums[0][:, h : h + 512],
                        lhsT=ident_sb[:],
                        rhs=ch_sb[:, 0, h : h + 512],
                        start=False,
                        stop=True,
                    )
                d_sb = io.tile([P, d], mybir.dt.float32, tag="dsb")
                nc.vector.tensor_tensor(
                    out=d_sb[:],
                    in0=ch_sb[:, 1, :],
                    in1=q_psums[1][:],
                    op=mybir.AluOpType.add,
                )
                t0 = SUP * s
                sq_tile = scratch.tile([P, d], mybir.dt.float32, tag="sq")
                nc.scalar.activation(
                    out=sq_tile[:],
                    in_=q_psums[0][:],
                    func=mybir.ActivationFunctionType.Square,
                    accum_out=d2_all[:, t0 : t0 + 1],
                )
                sq_tile2 = scratch.tile([P, d], mybir.dt.float32, tag="sq2")
                nc.scalar.activation(
                    out=sq_tile2[:],
                    in_=d_sb[:],
                    func=mybir.ActivationFunctionType.Square,
                    accum_out=d2_all[:, t0 + 1 : t0 + 2],
                )

            dist_all = persist.tile([P, nt], mybir.dt.float32)
            nc.scalar.activation(
                out=dist_all[:],
                in_=d2_all[:],
                func=mybir.ActivationFunctionType.Sqrt,
            )
            nc.scalar.activation(
                out=vals_sb[:],
                in_=dist_all[:],
                func=mybir.ActivationFunctionType.Exp,
                scale=1.0 / SMOOTH,
            )
            nc.sync.dma_start(out=vals.ap(), in_=vals_sb[:])
    nc.compile()
    return nc


def prep_v4(cw_sorted):
    widx_all, nsel_all = [], []
    neye = -np.eye(P, dtype=np.float16)
    for c in range(NCORES):
        shard = cw_sorted[c * CSH : (c + 1) * CSH]
        widx = np.empty((NSUP, P), np.int32)
        nsel = np.empty((NT, P, P), np.float16)
        for s in range(NSUP):
            seg = shard[s * SUPW : (s + 1) * SUPW]
            uw = np.unique(seg)
            if len(uw) > P:
                return None
            widx[s, : len(uw)] = uw
            widx[s, len(uw) :] = uw[-1]
            rel = np.searchsorted(uw, seg).reshape(SUP, P)
            for k in range(SUP):
                nsel[SUP * s + k] = neye[:, rel[k]]
        widx_all.append(np.ascontiguousarray(widx.T))
        nsel_all.append(nsel)
    return widx_all, nsel_all


def make_in_maps_v4(h_t, ch_sorted, widx_all, nsel_all):
    ident = np.eye(P, dtype=np.float16)
    in_maps = []
    for c in range(NCORES):
        sl = slice(c * CSH, (c + 1) * CSH)
        in_maps.append(
            {
                "ht": h_t,
                "ch": ch_sorted[sl].astype(np.float16),
                "widx": widx_all[c],
                "nsel": nsel_all[c],
                "ident": ident,
            }
        )
    return in_maps


def finish_on_host(vals_sorted, cw_sorted):
    """segment-sum + log_softmax (tiny O(C)+O(V) work)."""
    p = np.bincount(cw_sorted, weights=vals_sorted.astype(np.float64), minlength=V)
    m = p.max()
    lse = m + np.log(np.exp(p - m).sum())
    return (p - lse).astype(np.float32)[None, :]


def _prep(h_t, cache_h, cache_words):
    h_t = np.ascontiguousarray(np.asarray(h_t), dtype=np.float32)
    cache_h = np.ascontiguousarray(np.asarray(cache_h), dtype=np.float32)
    cw = np.asarray(cache_words).astype(np.int32)
    order = np.argsort(cw, kind="stable")
    return h_t, cache_h[order], cw[order]


def run_device(h_t, ch_sorted, cw_sorted, force_v1=False, verbose=False):
    """Compile + run the SPMD program; returns per-element vals (sorted order)."""
    import time as _time

    _t0 = _time.time()
    v7 = prep_v7(h_t, ch_sorted, cw_sorted)
    if v7 is not None:
        chb, wob, relf, b, plan, ndmax, woff_s, wtot = v7
        nc = build_nc_v7(plan, ndmax, woff_s, wtot)
        in_maps = make_in_maps_v7(chb, wob, relf)
        if verbose:
            mb = (chb.nbytes / NCORES + wob[0].nbytes + relf[0].nbytes) / 1e6
            print(f"[run_device] build+prep(v7): {_time.time() - _t0:.1f}s "
                  f"({mb:.1f} MB/core, wtot={wtot})")
        _t1 = _time.time()
        res = run_bass_kernel_spmd(nc, in_maps, core_ids=list(range(NCORES)))
        if verbose:
            print(f"[run_device] compile+exec: {_time.time() - _t1:.1f}s")
        # dsel[p, t] = selected -2*dot for element t*128+p (per core)
        dsel = np.concatenate(
            [r["dsel"].T.reshape(-1) for r in res.results]
        ).astype(np.float64)
        d2 = np.maximum(b + dsel, 0.0)
        return np.exp(np.sqrt(d2) / SMOOTH)

    v4 = prep_v4(cw_sorted)
    assert v4 is not None, "both v7 and v4 prep failed"
    nc = build_nc_v4()
    in_maps = make_in_maps_v4(h_t, ch_sorted, *v4)
    if verbose:
        print(f"[run_device] build+prep(v4): {_time.time() - _t0:.1f}s")
    _t1 = _time.time()
    res = run_bass_kernel_spmd(nc, in_maps, core_ids=list(range(NCORES)))
    if verbose:
        print(f"[run_device] compile+exec: {_time.time() - _t1:.1f}s")
    return np.concatenate([r["vals"].T.reshape(-1) for r in res.results])


def kernel(h_t, cache_h, cache_words):
    h_t, ch_sorted, cw_sorted = _prep(h_t, cache_h, cache_words)
    vals_sorted = run_device(h_t, ch_sorted, cw_sorted)
    return finish_on_host(vals_sorted, cw_sorted)
